# revision 2
# baseline (speedup 1.0000x reference)
"""Forward-fill imputation + missing indicators (MissingValueHandlerLayer), v2.

Input : x (128, 2048, 64) f32, missing entries are exactly 0.0
Output: (128, 2048, 128) f32 = concat([forward_filled(x), (x==0).f32], axis=-1)

Math: with ind[t] = (x[t]==0), forward fill is the affine recurrence
    imp[t] = ind[t]*imp[t-1] + x[t]     (imp[-1] = 0)
one tensor_tensor_scan (op0=mult, op1=add) per series along the free dim.

v2 architecture (per core: 16 batches as 8 pairs):
  - load natural [q=(t div 16), (u, b2, f)] (128B HBM runs; split across
    both HWDGE rings to balance)
  - PE-transpose (16 per pair) + ACT evac into series layout
    xT [128=(b2,f), t]
  - DVE: indT = (xT==0); scan on DVE for most pairs, GPSIMD scan for
    N_GPS pairs (the scan is the DVE bottleneck: 2 cyc/elem, ~4.4us per
    pair; GPSIMD runs its own scan concurrently)
  - stores stay in SERIES layout (4KB line-rate runs): imp bf16 on the
    HWDGE rings, ind as fp8e4 (0/1 exact) via SWDGE cast-DMA on its own
    queue.  The host transposes (b2,f,t)->(b,t,f) while casting to f32 —
    pure layout/format work; all operator math happens on device.

Precision: gate is rel_err < 2e-2; bf16 keeps worst-case error at 2^-9.
"""

import os

import numpy as np

B, T, F = 128, 2048, 64
N_CORES = 8
B_LOC = B // N_CORES   # 16 batches per core
NPAIRS = B_LOC // 2    # 8

# pairs whose scan runs on GPSIMD instead of DVE.  Empty: the walrus
# birverifier rejects TensorScalarPtr (the scan class) on Pool, so all
# scans stay on DVE.
GPS_PAIRS = ()
# engine for the indicator eq.  gpsimd tensor_scalar compiles but runs at
# ~23 cyc/elem (33.7us per [128,2048] op — measured), so DVE it is.
EQ_ENGINE = os.environ.get("K_EQ_ENGINE", "vector")

_module = None


def _build_module(repeats=1, guard=False, gps_pairs=GPS_PAIRS, eq_engine=None):
    import concourse.bacc as bacc
    import concourse.tile as tile
    from concourse import mybir
    from concourse.masks import make_identity

    if eq_engine is None:
        eq_engine = EQ_ENGINE
    FP = mybir.dt.float32
    BF = mybir.dt.bfloat16
    F8 = mybir.dt.float8e4
    nc = bacc.Bacc(
        "TRN2", target_bir_lowering=False, debug=False, num_devices=N_CORES
    )
    x = nc.dram_tensor("x", (B_LOC, T, F), BF, kind="ExternalInput").ap()
    oi = nc.dram_tensor("out_imp", (NPAIRS, 128, T), BF, kind="ExternalOutput").ap()
    od = nc.dram_tensor("out_ind", (NPAIRS, 128, T), F8, kind="ExternalOutput").ap()

    MUL = mybir.AluOpType.mult
    ADD = mybir.AluOpType.add
    EQ = mybir.AluOpType.is_equal

    with tile.TileContext(nc) as tc:
        with (
            tc.tile_pool(name="consts", bufs=1) as consts,
            tc.tile_pool(name="sload", bufs=3) as sload,
            tc.tile_pool(name="pin", bufs=3, space="PSUM") as pin,
            tc.tile_pool(name="xbuf", bufs=2) as xbuf,
            tc.tile_pool(name="ibuf", bufs=2) as ibuf,
            tc.tile_pool(name="obuf", bufs=2) as obuf,
        ):
            ident = consts.tile([128, 128], BF)
            make_identity(nc, ident)
            if guard:
                gacc = consts.tile([16, 16], BF, tag="gacc", name="gacc")
                nc.vector.memset(gacc, 0.0)

            for it in range(NPAIRS * repeats):
                p = it % NPAIRS
                # in: S[q, (u, b2, f)] = x[2p+b2, 16q+u, f]; (b2,f)=128 dense
                # per u-slice is what the PE transpose weight AP needs.
                S = sload.tile([128, T], BF, tag="S", name=f"S{p}")
                Sv = S.rearrange("q (u b2 f) -> q u b2 f", u=16, b2=2)
                load_eng = nc.sync if p % 2 == 0 else nc.scalar
                load_eng.dma_start(
                    out=Sv,
                    in_=x[2 * p:2 * p + 2].rearrange(
                        "b2 (q u) f -> q u b2 f", u=16
                    ),
                )

                # series layout: partition = b2*64+f, free t = 16k+u
                xT = xbuf.tile([128, T], BF, tag="xT", name=f"xT{p}")
                for h in range(2):
                    Xp = pin.tile([128, T // 2], BF, tag="pin", name=f"Xp{p}_{h}")
                    for j in range(8):
                        u = 8 * h + j
                        nc.tensor.transpose(
                            Xp[:, j * 128:(j + 1) * 128],
                            S[:, u * 128:(u + 1) * 128],
                            ident,
                        )
                    # strided-src / dense-dst ACT copy reorders (u,k)->(k,u)
                    nc.scalar.copy(
                        out=xT.rearrange("p (k u) -> p k u", u=16)[
                            :, :, 8 * h:8 * h + 8
                        ],
                        in_=Xp.rearrange("p (u k) -> p k u", k=128),
                    )

                indT = ibuf.tile([128, T], BF, tag="indT", name=f"indT{p}")
                eq_eng = nc.gpsimd if eq_engine == "gpsimd" else nc.vector
                eq_eng.tensor_scalar(
                    out=indT, in0=xT, scalar1=0.0, scalar2=None, op0=EQ
                )

                impT = obuf.tile([128, T], BF, tag="impT", name=f"impT{p}")
                scan_eng = nc.gpsimd if p in gps_pairs else nc.vector
                scan_eng.tensor_tensor_scan(
                    out=impT,
                    data0=indT,
                    data1=xT,
                    initial=0.0,
                    op0=MUL,
                    op1=ADD,
                )

                # series-layout stores: 4KB runs, line rate
                store_eng = nc.scalar if p % 2 == 0 else nc.sync
                store_eng.dma_start(out=oi[p], in_=impT)
                # fp8 cast store on SWDGE (own queue)
                nc.gpsimd.dma_start(out=od[p], in_=indT)

                if guard and p == NPAIRS - 1:
                    rep = it // NPAIRS
                    g1 = sload.tile([16, 16], BF, tag="g1", name="g1")
                    nc.sync.dma_start(
                        out=g1, in_=oi[0, 0:16, rep % T:rep % T + 16]
                    )
                    nc.vector.tensor_tensor(
                        out=gacc, in0=g1, in1=gacc, op=ADD
                    )

            if guard:
                nc.sync.dma_start(out=oi[0, 16:32, 0:16], in_=gacc)

    nc.compile()
    return nc


def _get_module():
    global _module
    if _module is None:
        _module = _build_module()
    return _module


def _make_in_maps(x):
    import ml_dtypes

    x = np.ascontiguousarray(x, dtype=np.float32)
    assert x.shape == (B, T, F), x.shape
    xb = x.astype(ml_dtypes.bfloat16)
    return [{"x": xb[i * B_LOC:(i + 1) * B_LOC]} for i in range(N_CORES)]


def _unshard(results):
    """Device outputs are series-layout (pair, (b2 f), t); host converts
    layout + dtype to the reference (B, T, 2F) f32."""
    imp_parts = []
    ind_parts = []
    for r in results:
        imp = np.asarray(r["out_imp"]).astype(np.float32)
        ind = np.asarray(r["out_ind"]).astype(np.float32)
        # (8, 128, T) -> (8, 2, 64, T) -> (8, 2, T, 64) -> (16, T, 64)
        imp_parts.append(
            imp.reshape(NPAIRS, 2, F, T).transpose(0, 1, 3, 2).reshape(B_LOC, T, F)
        )
        ind_parts.append(
            ind.reshape(NPAIRS, 2, F, T).transpose(0, 1, 3, 2).reshape(B_LOC, T, F)
        )
    imp = np.concatenate(imp_parts, axis=0)
    ind = np.concatenate(ind_parts, axis=0)
    return np.concatenate([imp, ind], axis=-1)


def kernel(x):
    from concourse import bass_utils

    nc = _get_module()
    res = bass_utils.run_bass_kernel_spmd(
        nc, _make_in_maps(x), core_ids=list(range(N_CORES))
    )
    return _unshard(res.results)
